# revision 1
# baseline (speedup 1.0000x reference)
"""2-layer peephole LSTM on 8 TRN2 NeuronCores.

Strategy (gate-dim sharding, full batch per core, pipelined layers):
- B=64, T=128, I=4096, H=1024. Each core owns a 128-wide slice of H (and the
  matching 512 = 4x128 gate columns per layer).
- Input projection xz0 = x @ Wx0_loc^T for all (t, b) is precomputed per core
  (phase A) into DRAM: out[bt, g] with bt t-major so each row-block of 64 rows
  is one timestep.
- Recurrence runs as 129 pipelined ticks: tick tau computes layer-0 step tau-1
  and layer-1 step tau-2. States are exchanged transposed ([slice,128] x [b,64])
  through one 8-rank AllGather per tick carrying [h0;c0;h1;c1] slices.
- All matmuls in f32r (1 cyc/row at N>=512): gate pre-acts accumulate in PSUM:
  sum_k U_chunk @ h_chunk + V_chunk @ c_chunk (V zero-padded on gate g) +
  ones x bias (K=1) + ident x xz_t (K=64 inject) [+ Wx1 @ h0 for layer 1].
"""
import sys
sys.path.insert(0, "/opt/trn_rl_repo")
import numpy as np
import concourse.bacc as bacc
import concourse.mybir as mybir
from concourse import tile

NC = 8
B, T, I, H = 64, 128, 4096, 1024
SL = H // NC          # 128 hidden cols per core
G = 4 * SL            # 512 gate cols per core
KC_I = I // 128       # 32 contraction chunks for phase A
KC_H = H // 128       # 8 contraction chunks for recurrence
F32 = mybir.dt.float32
F32R = mybir.dt.float32r
AF = mybir.ActivationFunctionType


def build(n_reps: int = 1):
    """Build the Bass program. n_reps>1 repeats the whole computation (for
    differential wall-clock timing)."""
    nc = bacc.Bacc("TRN2", target_bir_lowering=False, debug=False, num_devices=NC)

    # ---- I/O ----
    xT = nc.dram_tensor("xT", [I, T * B], F32R, kind="ExternalInput").ap()
    wx0 = nc.dram_tensor("wx0", [I, G], F32R, kind="ExternalInput").ap()
    u0 = nc.dram_tensor("u0", [H, G], F32R, kind="ExternalInput").ap()
    v0 = nc.dram_tensor("v0", [H, G], F32R, kind="ExternalInput").ap()
    wx1 = nc.dram_tensor("wx1", [H, G], F32R, kind="ExternalInput").ap()
    u1 = nc.dram_tensor("u1", [H, G], F32R, kind="ExternalInput").ap()
    v1 = nc.dram_tensor("v1", [H, G], F32R, kind="ExternalInput").ap()
    b0 = nc.dram_tensor("b0", [1, G], F32R, kind="ExternalInput").ap()
    b1 = nc.dram_tensor("b1", [1, G], F32R, kind="ExternalInput").ap()
    ones_in = nc.dram_tensor("ones_in", [1, B], F32R, kind="ExternalInput").ap()
    ident_r = nc.dram_tensor("ident_r", [B, B], F32R, kind="ExternalInput").ap()
    ident_f = nc.dram_tensor("ident_f", [B, B], F32, kind="ExternalInput").ap()
    seed = nc.dram_tensor("seed", [4 * H, B], F32, kind="ExternalInput").ap()
    init_own = nc.dram_tensor("init_own", [4 * SL, B], F32, kind="ExternalInput").ap()
    c0loc_in = nc.dram_tensor("c0loc_in", [2, B, SL], F32, kind="ExternalInput").ap()

    out_loc = nc.dram_tensor("out_loc", [T, B, SL], F32, kind="ExternalOutput").ap()
    hn_loc = nc.dram_tensor("hn_loc", [2, B, SL], F32, kind="ExternalOutput").ap()
    cn_loc = nc.dram_tensor("cn_loc", [2, B, SL], F32, kind="ExternalOutput").ap()

    with tile.TileContext(nc) as tc:
        with (
            tc.tile_pool(name="wpool", bufs=1) as wp,
            tc.tile_pool(name="dscr", bufs=1, space="DRAM") as dscr,
        ):
            # ---- resident weights in SBUF ----
            u0s = wp.tile([128, KC_H, G], F32R)
            nc.sync.dma_start(u0s[:], u0.rearrange("(c p) g -> p c g", p=128))
            v0s = wp.tile([128, KC_H, G], F32R)
            nc.sync.dma_start(v0s[:], v0.rearrange("(c p) g -> p c g", p=128))
            u1s = wp.tile([128, KC_H, G], F32R)
            nc.sync.dma_start(u1s[:], u1.rearrange("(c p) g -> p c g", p=128))
            v1s = wp.tile([128, KC_H, G], F32R)
            nc.sync.dma_start(v1s[:], v1.rearrange("(c p) g -> p c g", p=128))
            wx1s = wp.tile([128, KC_H, G], F32R)
            nc.sync.dma_start(wx1s[:], wx1.rearrange("(c p) g -> p c g", p=128))
            b0s = wp.tile([1, G], F32R)
            nc.sync.dma_start(b0s[:], b0)
            b1s = wp.tile([1, G], F32R)
            nc.sync.dma_start(b1s[:], b1)
            ones = wp.tile([1, B], F32R)
            nc.sync.dma_start(ones[:], ones_in)
            idr = wp.tile([B, B], F32R)
            nc.sync.dma_start(idr[:], ident_r)
            idf = wp.tile([B, B], F32)
            nc.sync.dma_start(idf[:], ident_f)

            xz0_store = dscr.tile([T * B, G], F32, name="xz0_store")

            for rep in range(n_reps):
                _one_pass(nc, tc, rep, u0s, v0s, u1s, v1s, wx1s, b0s, b1s, ones,
                          idr, idf, xT, wx0, seed, init_own, c0loc_in,
                          xz0_store, out_loc, hn_loc, cn_loc)
    nc.compile()
    return nc


def _one_pass(nc, tc, rep, u0s, v0s, u1s, v1s, wx1s, b0s, b1s, ones, idr, idf,
              xT, wx0, seed, init_own, c0loc_in, xz0_store,
              out_loc, hn_loc, cn_loc):
    F = mybir.ActivationFunctionType
    # ================= Phase A: xz0[bt, g] = x @ Wx0_loc^T =================
    with (
        tc.tile_pool(name=f"pa_sb{rep}", bufs=2) as pa,
        tc.tile_pool(name=f"pa_w{rep}", bufs=1) as paw,
        tc.tile_pool(name=f"pa_ps{rep}", bufs=4, space="PSUM") as pap,
    ):
        wx0s = paw.tile([128, KC_I, G], F32R, name="wx0s")
        nc.sync.dma_start(wx0s[:], wx0.rearrange("(c p) g -> p c g", p=128))
        for j in range(T * B // 128):  # 64 tiles of 128 bt-rows (= 2 steps)
            xstat = pa.tile([128, KC_I, 128], F32R, name="xstat", tag="xstat")
            nc.sync.dma_start(
                xstat[:], xT[:, j * 128:(j + 1) * 128].rearrange("(c p) m -> p c m", p=128)
            )
            psA = pap.tile([128, G], F32, name="psA", tag="psA")
            for ic in range(KC_I):
                nc.tensor.matmul(psA[:], xstat[:, ic, :], wx0s[:, ic, :],
                                 start=(ic == 0), stop=(ic == KC_I - 1))
            sA = pa.tile([128, G], F32, name="sA", tag="sA")
            nc.vector.tensor_copy(sA[:], psA[:])
            nc.sync.dma_start(xz0_store[j * 128:(j + 1) * 128, :], sA[:])

    # ================= Recurrence: 129 pipelined ticks =================
    with (
        tc.tile_pool(name=f"rec_sb{rep}", bufs=2) as rs,
        tc.tile_pool(name=f"rec_ps{rep}", bufs=2, space="PSUM") as rp,
        tc.tile_pool(name=f"rec_dram{rep}", bufs=2, space="DRAM") as rd,
    ):
        # seed gathered-state buffer
        agS = rs.tile([128, 4 * KC_H, B], F32, name="agS", tag="agS")
        nc.sync.dma_start(agS[:], seed.rearrange("(c p) b -> p c b", p=128))
        # local c slices (untransposed)
        c0loc = rs.tile([B, SL], F32, name="c0loc", tag="c0loc")
        nc.sync.dma_start(c0loc[:], c0loc_in[0])
        c1loc = rs.tile([B, SL], F32, name="c1loc", tag="c1loc")
        nc.sync.dma_start(c1loc[:], c0loc_in[1])

        # AG bounce for tick 1 pre-filled with own init slices (h1/c1 part)
        agin = rd.tile([4 * SL, B], F32, name="agin", tag="agin")
        nc.sync.dma_start(agin[2 * SL:4 * SL, :], init_own[2 * SL:4 * SL, :])

        for tau in range(1, T + 2):
            # -------- layer 0: step tau-1 --------
            if tau <= T:
                t0 = tau - 1
                xzt = rs.tile([B, G], F32, name="xzt", tag="xzt", bufs=3)
                nc.sync.dma_start(xzt[:], xz0_store[t0 * B:(t0 + 1) * B, :])
                ps0 = rp.tile([B, G], F32, name="ps0", tag="ps0")
                for k in range(KC_H):
                    nc.tensor.matmul(ps0[:], agS[:, 4 * k + 0, :].bitcast(F32R), u0s[:, k, :],
                                     start=(k == 0), stop=False)
                for k in range(KC_H):
                    nc.tensor.matmul(ps0[:], agS[:, 4 * k + 1, :].bitcast(F32R), v0s[:, k, :],
                                     start=False, stop=False)
                nc.tensor.matmul(ps0[:], ones[:], b0s[:], start=False, stop=False)
                nc.tensor.matmul(ps0[:], idr[:], xzt[:].bitcast(F32R), start=False, stop=True)
                new_h0, new_c0 = _gates(nc, rs, rp, ps0, c0loc, "l0", tau)
                c0loc = new_c0
                if tau == T:
                    nc.sync.dma_start(hn_loc[0], new_h0[:])
                    nc.sync.dma_start(cn_loc[0], new_c0[:])
            # -------- layer 1: step tau-2 --------
            if tau >= 2:
                t1 = tau - 2
                ps1 = rp.tile([B, G], F32, name="ps1", tag="ps1")
                for k in range(KC_H):
                    nc.tensor.matmul(ps1[:], agS[:, 4 * k + 0, :].bitcast(F32R), wx1s[:, k, :],
                                     start=(k == 0), stop=False)
                for k in range(KC_H):
                    nc.tensor.matmul(ps1[:], agS[:, 4 * k + 2, :].bitcast(F32R), u1s[:, k, :],
                                     start=False, stop=False)
                for k in range(KC_H):
                    nc.tensor.matmul(ps1[:], agS[:, 4 * k + 3, :].bitcast(F32R), v1s[:, k, :],
                                     start=False, stop=False)
                nc.tensor.matmul(ps1[:], ones[:], b1s[:], start=False, stop=True)
                new_h1, new_c1 = _gates(nc, rs, rp, ps1, c1loc, "l1", tau)
                c1loc = new_c1
                nc.sync.dma_start(out_loc[t1], new_h1[:])
                if tau == T + 1:
                    nc.sync.dma_start(hn_loc[1], new_h1[:])
                    nc.sync.dma_start(cn_loc[1], new_c1[:])

            # -------- transpose new slices + AllGather --------
            if tau <= T:
                pst = rp.tile([128, 4, B], F32, name="pst", tag="pst")
                nc.tensor.transpose(pst[:, 0, :], new_h0[:], idf[:])
                nc.tensor.transpose(pst[:, 1, :], new_c0[:], idf[:])
                if tau >= 2:
                    nc.tensor.transpose(pst[:, 2, :], new_h1[:], idf[:])
                    nc.tensor.transpose(pst[:, 3, :], new_c1[:], idf[:])
                sT = rs.tile([128, 4, B], F32, name="sT", tag="sT")
                nc.vector.tensor_copy(sT[:, 0:2, :] if tau == 1 else sT[:],
                                      pst[:, 0:2, :] if tau == 1 else pst[:])
                agin_t = rd.tile([4 * SL, B], F32, name="agin", tag="agin") if tau > 1 else agin
                if tau == 1:
                    nc.sync.dma_start(
                        agin_t[0:2 * SL].rearrange("(c p) b -> p c b", p=128),
                        sT[:, 0:2, :])
                else:
                    nc.sync.dma_start(
                        agin_t.rearrange("(c p) b -> p c b", p=128), sT[:])
                agout = rd.tile([4 * H, B], F32, name="agout", tag="agout",
                                addr_space="Shared")
                nc.gpsimd.collective_compute(
                    "AllGather", mybir.AluOpType.bypass,
                    replica_groups=[list(range(NC))],
                    ins=[agin_t.opt()], outs=[agout.opt()],
                )
                agS = rs.tile([128, 4 * KC_H, B], F32, name="agS", tag="agS")
                nc.sync.dma_start(agS[:], agout.rearrange("(c p) b -> p c b", p=128))


def _gates(nc, rs, rp, ps, cloc_old, lname, tau):
    """Gate nonlinearities + state update. ps: [B, G] pre-acts (i,f,g,o x SL).
    Returns (new_h [B,SL] sbuf, new_c [B,SL] sbuf)."""
    F = mybir.ActivationFunctionType
    gif = rs.tile([B, 2 * SL], F32, name=f"gif_{lname}", tag=f"gif_{lname}")
    nc.scalar.activation(gif[:], ps[:, 0:2 * SL], F.Sigmoid)
    gg = rs.tile([B, SL], F32, name=f"gg_{lname}", tag=f"gg_{lname}")
    nc.scalar.activation(gg[:], ps[:, 2 * SL:3 * SL], F.Tanh)
    go = rs.tile([B, SL], F32, name=f"go_{lname}", tag=f"go_{lname}")
    nc.scalar.activation(go[:], ps[:, 3 * SL:4 * SL], F.Sigmoid)
    t1 = rs.tile([B, SL], F32, name=f"t1_{lname}", tag=f"t1_{lname}")
    nc.vector.tensor_mul(t1[:], gif[:, 0:SL], gg[:])          # i*g
    new_c = rs.tile([B, SL], F32, name=f"c_{lname}", tag=f"c{lname}")
    nc.vector.tensor_mul(new_c[:], gif[:, SL:2 * SL], cloc_old[:])  # f*c
    nc.vector.tensor_add(new_c[:], new_c[:], t1[:])
    tc_ = rs.tile([B, SL], F32, name=f"tc_{lname}", tag=f"tc_{lname}")
    nc.scalar.activation(tc_[:], new_c[:], F.Tanh)
    new_h = rs.tile([B, SL], F32, name=f"h_{lname}", tag=f"h{lname}")
    nc.vector.tensor_mul(new_h[:], go[:], tc_[:])
    return new_h, new_c


# ===================== host-side prep / wrap =====================

def prep_in_maps(x, h0, c0, Wx0, Uh0, Vc0, b0, Wx1, Uh1, Vc1, b1):
    x = np.ascontiguousarray(x, dtype=np.float32)
    xT = np.ascontiguousarray(x.transpose(2, 1, 0).reshape(I, T * B))
    ident = np.eye(B, dtype=np.float32)
    ones = np.ones((1, B), np.float32)

    def slc(w, c):
        # w: [g, H, D] -> [D, 4*SL] for core c (gates stacked i,f,g,o)
        return np.ascontiguousarray(
            np.concatenate([w[g, c * SL:(c + 1) * SL, :] for g in range(w.shape[0])],
                           axis=0).T)

    def slc_v(v, c):
        # peephole [i,f,o] -> padded [i,f,0,o] -> [H, 4*SL]
        z = np.zeros((SL, H), np.float32)
        m = np.concatenate([v[0, c * SL:(c + 1) * SL, :], v[1, c * SL:(c + 1) * SL, :],
                            z, v[2, c * SL:(c + 1) * SL, :]], axis=0)
        return np.ascontiguousarray(m.T)

    def bias_s(b, c):
        return np.ascontiguousarray(
            np.concatenate([b[g, c * SL:(c + 1) * SL] for g in range(4)])[None, :])

    h0T = [np.ascontiguousarray(h0[l].T) for l in range(2)]  # [H, B]
    c0T = [np.ascontiguousarray(c0[l].T) for l in range(2)]
    seed = np.empty((4 * H, B), np.float32)
    for r in range(NC):
        seed[r * 4 * SL + 0 * SL: r * 4 * SL + 1 * SL] = h0T[0][r * SL:(r + 1) * SL]
        seed[r * 4 * SL + 1 * SL: r * 4 * SL + 2 * SL] = c0T[0][r * SL:(r + 1) * SL]
        seed[r * 4 * SL + 2 * SL: r * 4 * SL + 3 * SL] = h0T[1][r * SL:(r + 1) * SL]
        seed[r * 4 * SL + 3 * SL: r * 4 * SL + 4 * SL] = c0T[1][r * SL:(r + 1) * SL]

    in_maps = []
    for c in range(NC):
        init_own = seed[c * 4 * SL:(c + 1) * 4 * SL]
        c0loc = np.stack([np.ascontiguousarray(c0[0][:, c * SL:(c + 1) * SL]),
                          np.ascontiguousarray(c0[1][:, c * SL:(c + 1) * SL])])
        in_maps.append(dict(
            xT=xT, wx0=slc(Wx0, c), u0=slc(Uh0, c), v0=slc_v(Vc0, c),
            wx1=slc(Wx1, c), u1=slc(Uh1, c), v1=slc_v(Vc1, c),
            b0=bias_s(b0, c), b1=bias_s(b1, c),
            ones_in=ones, ident_r=ident, ident_f=ident,
            seed=seed, init_own=np.ascontiguousarray(init_own), c0loc_in=c0loc,
        ))
    return in_maps


def assemble(results):
    """results: list (per core) of dicts with out_loc/hn_loc/cn_loc."""
    out = np.concatenate([r["out_loc"] for r in results], axis=2)  # [T, B, H]
    out = np.ascontiguousarray(out.transpose(1, 0, 2))             # [B, T, H]
    h_n = np.concatenate([r["hn_loc"] for r in results], axis=2)   # [2, B, H]
    c_n = np.concatenate([r["cn_loc"] for r in results], axis=2)
    return out, h_n, c_n


# ===================== harness entry point =====================

def kernel(**inputs):
    """Full-input entry: shards across 8 NeuronCores internally, returns
    (output [B,T,H], h_n [2,B,H], c_n [2,B,H]) matching the reference."""
    from concourse.bass_utils import run_bass_kernel_spmd
    global _NC_CACHE
    try:
        nc = _NC_CACHE
    except NameError:
        nc = _NC_CACHE = build(n_reps=1)
    in_maps = prep_in_maps(
        x=np.asarray(inputs["x"], np.float32),
        h0=np.asarray(inputs["h0"], np.float32),
        c0=np.asarray(inputs["c0"], np.float32),
        Wx0=np.asarray(inputs["Wx0"], np.float32),
        Uh0=np.asarray(inputs["Uh0"], np.float32),
        Vc0=np.asarray(inputs["Vc0"], np.float32),
        b0=np.asarray(inputs["b0"], np.float32),
        Wx1=np.asarray(inputs["Wx1"], np.float32),
        Uh1=np.asarray(inputs["Uh1"], np.float32),
        Vc1=np.asarray(inputs["Vc1"], np.float32),
        b1=np.asarray(inputs["b1"], np.float32),
    )
    res = run_bass_kernel_spmd(nc, in_maps, core_ids=list(range(NC)))
    return assemble(res.results)


# revision 2
# speedup vs baseline: 1.0512x; 1.0512x over previous
"""2-layer peephole LSTM on 8 TRN2 NeuronCores.

Strategy (gate-dim sharding, full batch per core, pipelined layers):
- B=64, T=128, I=4096, H=1024. Each core owns a 128-wide slice of H (and the
  matching 512 = 4x128 gate columns per layer).
- Input projection xz0 = x @ Wx0_loc^T for all (t, b) is precomputed per core
  (phase A) into DRAM: out[bt, g] with bt t-major so each row-block of 64 rows
  is one timestep.
- Recurrence runs as 129 pipelined ticks: tick tau computes layer-0 step tau-1
  and layer-1 step tau-2. States are exchanged transposed ([slice,128] x [b,64])
  through one 8-rank AllGather per tick carrying [h0;c0;h1;c1] slices.
- All matmuls in f32r (1 cyc/row at N>=512): gate pre-acts accumulate in PSUM:
  sum_k U_chunk @ h_chunk + V_chunk @ c_chunk (V zero-padded on gate g) +
  ones x bias (K=1) + ident x xz_t (K=64 inject) [+ Wx1 @ h0 for layer 1].
"""
import sys
sys.path.insert(0, "/opt/trn_rl_repo")
import numpy as np
import concourse.bacc as bacc
import concourse.mybir as mybir
from concourse import tile

NC = 8
B, T, I, H = 64, 128, 4096, 1024
SL = H // NC          # 128 hidden cols per core
G = 4 * SL            # 512 gate cols per core
KC_I = I // 128       # 32 contraction chunks for phase A
KC_H = H // 128       # 8 contraction chunks for recurrence
F32 = mybir.dt.float32
F32R = mybir.dt.float32r
AF = mybir.ActivationFunctionType


def build(n_reps: int = 1):
    """Build the Bass program. n_reps>1 repeats the whole computation (for
    differential wall-clock timing)."""
    nc = bacc.Bacc("TRN2", target_bir_lowering=False, debug=False, num_devices=NC)

    # ---- I/O ----
    xT = nc.dram_tensor("xT", [I, T * B], F32R, kind="ExternalInput").ap()
    wx0 = nc.dram_tensor("wx0", [I, G], F32R, kind="ExternalInput").ap()
    u0 = nc.dram_tensor("u0", [H, G], F32R, kind="ExternalInput").ap()
    v0 = nc.dram_tensor("v0", [H, G], F32R, kind="ExternalInput").ap()
    wx1 = nc.dram_tensor("wx1", [H, G], F32R, kind="ExternalInput").ap()
    u1 = nc.dram_tensor("u1", [H, G], F32R, kind="ExternalInput").ap()
    v1 = nc.dram_tensor("v1", [H, G], F32R, kind="ExternalInput").ap()
    b0 = nc.dram_tensor("b0", [1, G], F32R, kind="ExternalInput").ap()
    b1 = nc.dram_tensor("b1", [1, G], F32R, kind="ExternalInput").ap()
    ones_in = nc.dram_tensor("ones_in", [1, B], F32R, kind="ExternalInput").ap()
    ident_r = nc.dram_tensor("ident_r", [B, B], F32R, kind="ExternalInput").ap()
    ident_f = nc.dram_tensor("ident_f", [B, B], F32, kind="ExternalInput").ap()
    seed = nc.dram_tensor("seed", [4 * H, B], F32, kind="ExternalInput").ap()
    init_own = nc.dram_tensor("init_own", [4 * SL, B], F32R, kind="ExternalInput").ap()
    c0loc_in = nc.dram_tensor("c0loc_in", [2, B, SL], F32, kind="ExternalInput").ap()

    out_loc = nc.dram_tensor("out_loc", [T, B, SL], F32, kind="ExternalOutput").ap()
    hn_loc = nc.dram_tensor("hn_loc", [2, B, SL], F32, kind="ExternalOutput").ap()
    cn_loc = nc.dram_tensor("cn_loc", [2, B, SL], F32, kind="ExternalOutput").ap()

    with tile.TileContext(nc) as tc:
        with (
            tc.tile_pool(name="wpool", bufs=1) as wp,
            tc.tile_pool(name="dscr", bufs=1, space="DRAM") as dscr,
        ):
            # ---- resident weights in SBUF ----
            u0s = wp.tile([128, KC_H, G], F32R)
            nc.sync.dma_start(u0s[:], u0.rearrange("(c p) g -> p c g", p=128))
            v0s = wp.tile([128, KC_H, G], F32R)
            nc.sync.dma_start(v0s[:], v0.rearrange("(c p) g -> p c g", p=128))
            u1s = wp.tile([128, KC_H, G], F32R)
            nc.sync.dma_start(u1s[:], u1.rearrange("(c p) g -> p c g", p=128))
            v1s = wp.tile([128, KC_H, G], F32R)
            nc.sync.dma_start(v1s[:], v1.rearrange("(c p) g -> p c g", p=128))
            wx1s = wp.tile([128, KC_H, G], F32R)
            nc.sync.dma_start(wx1s[:], wx1.rearrange("(c p) g -> p c g", p=128))
            b0s = wp.tile([1, G], F32R)
            nc.sync.dma_start(b0s[:], b0)
            b1s = wp.tile([1, G], F32R)
            nc.sync.dma_start(b1s[:], b1)
            ones = wp.tile([1, B], F32R)
            nc.sync.dma_start(ones[:], ones_in)
            idr = wp.tile([B, B], F32R)
            nc.sync.dma_start(idr[:], ident_r)
            idf = wp.tile([B, B], F32)
            nc.sync.dma_start(idf[:], ident_f)

            xz0_store = dscr.tile([T * B, G], F32R, name="xz0_store")

            for rep in range(n_reps):
                _one_pass(nc, tc, rep, u0s, v0s, u1s, v1s, wx1s, b0s, b1s, ones,
                          idr, idf, xT, wx0, seed, init_own, c0loc_in,
                          xz0_store, out_loc, hn_loc, cn_loc)
    nc.compile()
    return nc


def _one_pass(nc, tc, rep, u0s, v0s, u1s, v1s, wx1s, b0s, b1s, ones, idr, idf,
              xT, wx0, seed, init_own, c0loc_in, xz0_store,
              out_loc, hn_loc, cn_loc):
    F = mybir.ActivationFunctionType
    # ================= Phase A: xz0[bt, g] = x @ Wx0_loc^T =================
    with (
        tc.tile_pool(name=f"pa_sb{rep}", bufs=2) as pa,
        tc.tile_pool(name=f"pa_w{rep}", bufs=1) as paw,
        tc.tile_pool(name=f"pa_ps{rep}", bufs=4, space="PSUM") as pap,
    ):
        wx0s = paw.tile([128, KC_I, G], F32R, name="wx0s")
        nc.sync.dma_start(wx0s[:], wx0.rearrange("(c p) g -> p c g", p=128))
        for j in range(T * B // 128):  # 64 tiles of 128 bt-rows (= 2 steps)
            xstat = pa.tile([128, KC_I, 128], F32R, name="xstat", tag="xstat")
            nc.sync.dma_start(
                xstat[:], xT[:, j * 128:(j + 1) * 128].rearrange("(c p) m -> p c m", p=128)
            )
            psA = pap.tile([128, G], F32, name="psA", tag="psA")
            for ic in range(KC_I):
                nc.tensor.matmul(psA[:], xstat[:, ic, :], wx0s[:, ic, :],
                                 start=(ic == 0), stop=(ic == KC_I - 1))
            sA = pa.tile([128, G], F32R, name="sA", tag="sA")
            nc.vector.tensor_copy(sA[:], psA[:])
            nc.sync.dma_start(xz0_store[j * 128:(j + 1) * 128, :], sA[:])

    # ================= Recurrence: 129 pipelined ticks =================
    with (
        tc.tile_pool(name=f"rec_sb{rep}", bufs=2) as rs,
        tc.tile_pool(name=f"rec_ps{rep}", bufs=2, space="PSUM") as rp,
        tc.tile_pool(name=f"rec_dram{rep}", bufs=2, space="DRAM") as rd,
    ):
        # seed gathered-state buffer
        agS = rs.tile([128, 4 * KC_H, B], F32R, name="agS", tag="agS")
        nc.gpsimd.dma_start(agS[:], seed.rearrange("(c p) b -> p c b", p=128))
        # local c slices (untransposed)
        c0loc = rs.tile([B, SL], F32, name="c0loc", tag="c0loc")
        nc.sync.dma_start(c0loc[:], c0loc_in[0])
        c1loc = rs.tile([B, SL], F32, name="c1loc", tag="c1loc")
        nc.sync.dma_start(c1loc[:], c0loc_in[1])

        # AG bounce for tick 1 pre-filled with own init slices (h1/c1 part)
        agin = rd.tile([4 * SL, B], F32R, name="agin", tag="agin")
        nc.sync.dma_start(agin[2 * SL:4 * SL, :], init_own[2 * SL:4 * SL, :])

        for tau in range(1, T + 2):
            # -------- layer 0: step tau-1 --------
            if tau <= T:
                t0 = tau - 1
                xzt = rs.tile([B, G], F32R, name="xzt", tag="xzt", bufs=3)
                nc.sync.dma_start(xzt[:], xz0_store[t0 * B:(t0 + 1) * B, :])
                ps0 = rp.tile([B, G], F32, name="ps0", tag="ps0")
                for k in range(KC_H):
                    nc.tensor.matmul(ps0[:], agS[:, 4 * k + 0, :], u0s[:, k, :],
                                     start=(k == 0), stop=False)
                for k in range(KC_H):
                    nc.tensor.matmul(ps0[:], agS[:, 4 * k + 1, :], v0s[:, k, :],
                                     start=False, stop=False)
                nc.tensor.matmul(ps0[:], ones[:], b0s[:], start=False, stop=False)
                nc.tensor.matmul(ps0[:], idr[:], xzt[:], start=False, stop=True)
                new_h0, new_c0 = _gates(nc, rs, rp, ps0, c0loc, "l0", tau)
                c0loc = new_c0
                if tau == T:
                    nc.sync.dma_start(hn_loc[0], new_h0[:])
                    nc.sync.dma_start(cn_loc[0], new_c0[:])
            # -------- layer 1: step tau-2 --------
            if tau >= 2:
                t1 = tau - 2
                ps1 = rp.tile([B, G], F32, name="ps1", tag="ps1")
                for k in range(KC_H):
                    nc.tensor.matmul(ps1[:], agS[:, 4 * k + 0, :], wx1s[:, k, :],
                                     start=(k == 0), stop=False)
                for k in range(KC_H):
                    nc.tensor.matmul(ps1[:], agS[:, 4 * k + 2, :], u1s[:, k, :],
                                     start=False, stop=False)
                for k in range(KC_H):
                    nc.tensor.matmul(ps1[:], agS[:, 4 * k + 3, :], v1s[:, k, :],
                                     start=False, stop=False)
                nc.tensor.matmul(ps1[:], ones[:], b1s[:], start=False, stop=True)
                new_h1, new_c1 = _gates(nc, rs, rp, ps1, c1loc, "l1", tau)
                c1loc = new_c1
                nc.sync.dma_start(out_loc[t1], new_h1[:])
                if tau == T + 1:
                    nc.sync.dma_start(hn_loc[1], new_h1[:])
                    nc.sync.dma_start(cn_loc[1], new_c1[:])

            # -------- transpose new slices + AllGather --------
            if tau <= T:
                srcs = [new_h0, new_c0] + ([new_h1, new_c1] if tau >= 2 else [])
                sT = rs.tile([128, 4, B], F32R, name="sT", tag="sT")
                for q, src in enumerate(srcs):
                    pst = rp.tile([128, B], F32, name=f"pst{q}", tag=f"pst{q}", bufs=1)
                    nc.tensor.transpose(pst[:], src[:], idf[:])
                    nc.vector.tensor_copy(sT[:, q, :], pst[:])
                agin_t = rd.tile([4 * SL, B], F32R, name="agin", tag="agin") if tau > 1 else agin
                if tau == 1:
                    nc.sync.dma_start(
                        agin_t[0:2 * SL].rearrange("(c p) b -> p c b", p=128),
                        sT[:, 0:2, :])
                else:
                    nc.sync.dma_start(
                        agin_t.rearrange("(c p) b -> p c b", p=128), sT[:])
                agout = rd.tile([4 * H, B], F32R, name="agout", tag="agout",
                                addr_space="Shared")
                nc.gpsimd.collective_compute(
                    "AllGather", mybir.AluOpType.bypass,
                    replica_groups=[list(range(NC))],
                    ins=[agin_t.opt()], outs=[agout.opt()],
                )
                agS = rs.tile([128, 4 * KC_H, B], F32R, name="agS", tag="agS")
                ag_v = agout.rearrange("(c p) b -> p c b", p=128)
                for q in range(4):
                    nc.sync.dma_start(agS[:, q * 8:(q + 1) * 8, :], ag_v[:, q * 8:(q + 1) * 8, :])


def _gates(nc, rs, rp, ps, cloc_old, lname, tau):
    """Gate nonlinearities + state update. ps: [B, G] pre-acts (i,f,g,o x SL).
    Returns (new_h [B,SL] sbuf, new_c [B,SL] sbuf)."""
    F = mybir.ActivationFunctionType
    gif = rs.tile([B, 2 * SL], F32, name=f"gif_{lname}", tag=f"gif_{lname}")
    nc.scalar.activation(gif[:], ps[:, 0:2 * SL], F.Sigmoid)
    gg = rs.tile([B, SL], F32, name=f"gg_{lname}", tag=f"gg_{lname}")
    nc.scalar.activation(gg[:], ps[:, 2 * SL:3 * SL], F.Tanh)
    go = rs.tile([B, SL], F32, name=f"go_{lname}", tag=f"go_{lname}")
    nc.scalar.activation(go[:], ps[:, 3 * SL:4 * SL], F.Sigmoid)
    t1 = rs.tile([B, SL], F32, name=f"t1_{lname}", tag=f"t1_{lname}")
    nc.vector.tensor_mul(t1[:], gif[:, 0:SL], gg[:])          # i*g
    new_c = rs.tile([B, SL], F32, name=f"c_{lname}", tag=f"c{lname}")
    nc.vector.tensor_mul(new_c[:], gif[:, SL:2 * SL], cloc_old[:])  # f*c
    nc.vector.tensor_add(new_c[:], new_c[:], t1[:])
    tc_ = rs.tile([B, SL], F32, name=f"tc_{lname}", tag=f"tc_{lname}")
    nc.scalar.activation(tc_[:], new_c[:], F.Tanh)
    new_h = rs.tile([B, SL], F32, name=f"h_{lname}", tag=f"h{lname}")
    nc.vector.tensor_mul(new_h[:], go[:], tc_[:])
    return new_h, new_c


# ===================== host-side prep / wrap =====================

def prep_in_maps(x, h0, c0, Wx0, Uh0, Vc0, b0, Wx1, Uh1, Vc1, b1):
    x = np.ascontiguousarray(x, dtype=np.float32)
    xT = np.ascontiguousarray(x.transpose(2, 1, 0).reshape(I, T * B))
    ident = np.eye(B, dtype=np.float32)
    ones = np.ones((1, B), np.float32)

    def slc(w, c):
        # w: [g, H, D] -> [D, 4*SL] for core c (gates stacked i,f,g,o)
        return np.ascontiguousarray(
            np.concatenate([w[g, c * SL:(c + 1) * SL, :] for g in range(w.shape[0])],
                           axis=0).T)

    def slc_v(v, c):
        # peephole [i,f,o] -> padded [i,f,0,o] -> [H, 4*SL]
        z = np.zeros((SL, H), np.float32)
        m = np.concatenate([v[0, c * SL:(c + 1) * SL, :], v[1, c * SL:(c + 1) * SL, :],
                            z, v[2, c * SL:(c + 1) * SL, :]], axis=0)
        return np.ascontiguousarray(m.T)

    def bias_s(b, c):
        return np.ascontiguousarray(
            np.concatenate([b[g, c * SL:(c + 1) * SL] for g in range(4)])[None, :])

    h0T = [np.ascontiguousarray(h0[l].T) for l in range(2)]  # [H, B]
    c0T = [np.ascontiguousarray(c0[l].T) for l in range(2)]
    seed = np.empty((4 * H, B), np.float32)
    for r in range(NC):
        seed[r * 4 * SL + 0 * SL: r * 4 * SL + 1 * SL] = h0T[0][r * SL:(r + 1) * SL]
        seed[r * 4 * SL + 1 * SL: r * 4 * SL + 2 * SL] = c0T[0][r * SL:(r + 1) * SL]
        seed[r * 4 * SL + 2 * SL: r * 4 * SL + 3 * SL] = h0T[1][r * SL:(r + 1) * SL]
        seed[r * 4 * SL + 3 * SL: r * 4 * SL + 4 * SL] = c0T[1][r * SL:(r + 1) * SL]

    in_maps = []
    for c in range(NC):
        init_own = seed[c * 4 * SL:(c + 1) * 4 * SL]
        c0loc = np.stack([np.ascontiguousarray(c0[0][:, c * SL:(c + 1) * SL]),
                          np.ascontiguousarray(c0[1][:, c * SL:(c + 1) * SL])])
        in_maps.append(dict(
            xT=xT, wx0=slc(Wx0, c), u0=slc(Uh0, c), v0=slc_v(Vc0, c),
            wx1=slc(Wx1, c), u1=slc(Uh1, c), v1=slc_v(Vc1, c),
            b0=bias_s(b0, c), b1=bias_s(b1, c),
            ones_in=ones, ident_r=ident, ident_f=ident,
            seed=seed, init_own=np.ascontiguousarray(init_own), c0loc_in=c0loc,
        ))
    return in_maps


def assemble(results):
    """results: list (per core) of dicts with out_loc/hn_loc/cn_loc."""
    out = np.concatenate([r["out_loc"] for r in results], axis=2)  # [T, B, H]
    out = np.ascontiguousarray(out.transpose(1, 0, 2))             # [B, T, H]
    h_n = np.concatenate([r["hn_loc"] for r in results], axis=2)   # [2, B, H]
    c_n = np.concatenate([r["cn_loc"] for r in results], axis=2)
    return out, h_n, c_n


# ===================== harness entry point =====================

_NC_CACHE = None


def kernel(**inputs):
    """Full-input entry: shards across 8 NeuronCores internally, returns
    (output [B,T,H], h_n [2,B,H], c_n [2,B,H]) matching the reference."""
    from concourse.bass_utils import run_bass_kernel_spmd
    global _NC_CACHE
    if _NC_CACHE is None:
        _NC_CACHE = build(n_reps=1)
    nc = _NC_CACHE
    in_maps = prep_in_maps(
        x=np.asarray(inputs["x"], np.float32),
        h0=np.asarray(inputs["h0"], np.float32),
        c0=np.asarray(inputs["c0"], np.float32),
        Wx0=np.asarray(inputs["Wx0"], np.float32),
        Uh0=np.asarray(inputs["Uh0"], np.float32),
        Vc0=np.asarray(inputs["Vc0"], np.float32),
        b0=np.asarray(inputs["b0"], np.float32),
        Wx1=np.asarray(inputs["Wx1"], np.float32),
        Uh1=np.asarray(inputs["Uh1"], np.float32),
        Vc1=np.asarray(inputs["Vc1"], np.float32),
        b1=np.asarray(inputs["b1"], np.float32),
    )
    res = run_bass_kernel_spmd(nc, in_maps, core_ids=list(range(NC)))
    return assemble(res.results)
